# revision 7
# baseline (speedup 1.0000x reference)
"""Trainium2 Bass kernel for nn_DirectedAcyclicDecoder (sparse banded attention).

Contract: kernel(**inputs) takes FULL unsharded numpy inputs, returns the FULL
[B, T, T] float32 output. Internally shards batch across 8 NeuronCores (one
example per core) and runs a fused Bass/Tile kernel per core.

Math (per batch b, fused form — validated against the jax reference):
  f   = concat(features, pos_embed)           [T, 2D]
  q   = f @ Wq.T + bq ; k = f @ Wk.T + bk     [T, D], heads of CH=64
  ge  = exp(f @ Wg.T + bg)                    [T, H]   (unnormalized gates)
  raw_h[i,j] = q_h[i] . k_h[j]                (banded: j in (i, n_valid))
  E_h = exp(raw_h/8 + mask)  ; Z_h[i] = sum_j E_h[i,j]
  out[i,j] = ln( sum_h (ge_h[i]/Z_h[i]) * E_h[i,j] / sum_h ge_h[i] )
invalid positions are fixed to -inf on the host during unshard.

All f32 matmul inputs ride in one [128, WF] f32 bundle DMA and the bf16 mask
constants in one [128, WB] bf16 bundle DMA, so every matmul needs at most one
semaphore wait (the PE LDWEIGHTS slot only has one).
"""

import os
import sys

import numpy as np

for _p in ("/opt/trn_rl_repo",):
    if _p not in sys.path:
        sys.path.insert(0, _p)

import ml_dtypes  # noqa: E402

import concourse.bass as bass  # noqa: E402
import concourse.bacc as bacc  # noqa: E402
import concourse.tile as tile  # noqa: E402
from concourse import mybir  # noqa: E402
from concourse.bass_utils import run_bass_kernel_spmd  # noqa: E402

B, T, D, H, CH = 8, 1024, 512, 8, 64
D2 = 2 * D            # 1024, contraction dim of the projections
KC = D2 // 128        # 8 contraction chunks
MT = D // 128         # 4 output chunks for q/k
PB = T // 128         # 8 query-position blocks of 128
NEG = np.float32(-8e30)   # additive mask on raw scores (pre /8 scale)

# f32 bundle column layout
OF_FT = 0                 # ft chunks, c-major          [8 x 1024]
OF_WQ = OF_FT + KC * T    # wq chunks, c-major          [8 x 512]
OF_WK = OF_WQ + KC * D    # wk chunks, c-major          [8 x 512]
OF_BQ = OF_WK + KC * D    # bq per-partition            [4]
OF_BK = OF_BQ + MT        # bk per-partition            [4]
OF_GE = OF_BK + MT        # exp(gate logits), ib-major  [8 x 8]
OF_RSG = OF_GE + PB * H   # 1/sum_h ge, ib-major        [8]
WF = OF_RSG + PB

# bf16 bundle column layout
OB_ID = 0                 # identity 128
OB_TRI = OB_ID + 128      # strict-upper-triangular additive mask
OB_CM = OB_TRI + 128      # column mask row (replicated on all partitions)
OB_ONE = OB_CM + T        # ones
WB = OB_ONE + 128

F32 = mybir.dt.float32
F32R = mybir.dt.float32r
BF16 = mybir.dt.bfloat16

_NC_CACHE: dict = {}
LAST_RESULTS = None   # BassKernelResults of the last run (for test harness)


def _build_nc(w_col: int) -> "bacc.Bacc":
    """Build the per-core Bass module.

    w_col: number of rightmost score columns needing the data-driven column
           mask (= T - min_b n_valid[b]); 0 compiles the mask matmul out.
    """
    nc = bacc.Bacc("TRN2", target_bir_lowering=False)
    A = mybir.ActivationFunctionType

    bf_d = nc.dram_tensor("bundle_f", [128, WF], F32R, kind="ExternalInput")
    bb_d = nc.dram_tensor("bundle_b", [128, WB], BF16, kind="ExternalInput")
    out_d = nc.dram_tensor("out", [T, T], F32, kind="ExternalOutput")

    with tile.TileContext(nc) as tc:
        with (
            tc.tile_pool(name="persist", bufs=1) as persist,
            tc.tile_pool(name="qk", bufs=1) as qkp,
            tc.tile_pool(name="psum_proj", bufs=4, space="PSUM") as pproj,
            tc.tile_pool(name="psum_scores", bufs=2, space="PSUM") as pscore,
            tc.tile_pool(name="epool", bufs=10) as epool,
            tc.tile_pool(name="accpool", bufs=2) as accpool,
            tc.tile_pool(name="linkpool", bufs=2) as linkpool,
            tc.tile_pool(name="small", bufs=4) as small,
        ):
            # ---- loads: one DMA per dtype bundle ------------------------
            bf_sb = persist.tile([128, WF], F32R, tag="bf")
            nc.sync.dma_start(out=bf_sb, in_=bf_d[:])
            bb_sb = persist.tile([128, WB], BF16, tag="bb")
            nc.sync.dma_start(out=bb_sb, in_=bb_d[:])

            ft_sb = bf_sb[:, OF_FT : OF_FT + KC * T].rearrange(
                "p (c t) -> p c t", t=T
            )
            wq_sb = bf_sb[:, OF_WQ : OF_WQ + KC * D].rearrange(
                "p (c m) -> p c m", m=D
            )
            wk_sb = bf_sb[:, OF_WK : OF_WK + KC * D].rearrange(
                "p (c m) -> p c m", m=D
            )
            bq_sb = bf_sb[:, OF_BQ : OF_BQ + MT].bitcast(F32)
            bk_sb = bf_sb[:, OF_BK : OF_BK + MT].bitcast(F32)
            ge_sb = bf_sb[:, OF_GE : OF_GE + PB * H].bitcast(F32).rearrange(
                "p (b h) -> p b h", h=H
            )
            rsg_sb = bf_sb[:, OF_RSG : OF_RSG + PB].bitcast(F32)
            ident_sb = bb_sb[:, OB_ID : OB_ID + 128]
            tri_sb = bb_sb[:, OB_TRI : OB_TRI + 128]
            cmask_sb = bb_sb[0:1, OB_CM : OB_CM + T]
            ones1_sb = bb_sb[0:1, OB_ONE : OB_ONE + 128]

            # ACT observes the bundle DMA queue once up front so later ACT
            # ops (which also depend on PE/DVE) never need a second wait slot
            # (engine instructions have a single sync-wait command).
            scratch = small.tile([128, 1], F32, tag="scratch")
            nc.scalar.copy(scratch, bq_sb[:, 0:1])

            # ---- projections: qt/kt [d_out, t] = W @ f.T ---------------
            qt_sb = qkp.tile([128, MT, T], F32R, tag="qt")
            kt_sb = qkp.tile([128, MT, T], F32R, tag="kt")
            for w_sb, o_sb, b_sb in ((wq_sb, qt_sb, bq_sb), (wk_sb, kt_sb, bk_sb)):
                for m in range(MT):
                    for nt in range(T // 512):
                        ps = pproj.tile([128, 512], F32, tag="proj")
                        for c in range(KC):
                            nc.tensor.matmul(
                                ps,
                                lhsT=w_sb[:, c, m * 128 : (m + 1) * 128],
                                rhs=ft_sb[:, c, nt * 512 : (nt + 1) * 512],
                                start=(c == 0),
                                stop=(c == KC - 1),
                            )
                        # PSUM -> SBUF copy with free per-partition bias add
                        nc.scalar.activation(
                            out=o_sb[:, m, nt * 512 : (nt + 1) * 512],
                            in_=ps,
                            func=A.Identity,
                            bias=b_sb[:, m : m + 1],
                        )

            # ---- per query-position block ------------------------------
            for ib in range(PB):
                i0 = ib * 128
                nj = T - i0
                e_tiles = []
                z_sb = small.tile([128, H], F32, tag="z")
                for h in range(H):
                    m = h // 2
                    r0 = 64 * (h % 2)
                    ps = pscore.tile([128, 1024], F32, tag="scores")
                    # QK^T for this head over the banded j range [i0, T)
                    for j0 in range(0, nj, 512):
                        jw = min(512, nj - j0)
                        nc.tensor.matmul(
                            ps[:, j0 : j0 + jw],
                            lhsT=qt_sb[r0 : r0 + 64, m, i0 : i0 + 128],
                            rhs=kt_sb[r0 : r0 + 64, m, i0 + j0 : i0 + j0 + jw],
                            start=True,
                            stop=False,
                            skip_group_check=True,
                        )
                    # triangular mask on the diagonal 128 columns:
                    # psum[:, :128] += I.T @ trimask
                    nc.tensor.matmul(
                        ps[:, 0:128],
                        lhsT=ident_sb,
                        rhs=tri_sb,
                        start=False,
                        stop=(w_col == 0),
                        skip_group_check=True,
                    )
                    if w_col > 0:
                        # column mask for j >= min n_valid: += ones1.T @ colmask
                        nc.tensor.matmul(
                            ps[:, nj - w_col : nj],
                            lhsT=ones1_sb,
                            rhs=cmask_sb[:, T - w_col : T],
                            start=False,
                            stop=True,
                            skip_group_check=True,
                        )
                    # E = exp(raw/8), Z = row-sum(E) fused on ACT
                    e_sb = epool.tile([128, 1024], F32, tag="E")
                    nc.scalar.activation(
                        out=e_sb[:, :nj],
                        in_=ps[:, :nj],
                        func=A.Exp,
                        scale=0.125,
                        accum_out=z_sb[:, h : h + 1],
                    )
                    e_tiles.append(e_sb)

                # w_h = ge_h / (Z_h + eps)   [128, H]
                zr_sb = small.tile([128, H], F32, tag="zr")
                nc.vector.tensor_scalar_add(zr_sb, z_sb, 1e-30)
                nc.vector.reciprocal(zr_sb, zr_sb)
                w_sb2 = small.tile([128, H], F32, tag="w")
                nc.vector.tensor_tensor(
                    w_sb2, ge_sb[:, ib, :], zr_sb, mybir.AluOpType.mult
                )

                # acc = sum_h w_h * E_h
                acc = accpool.tile([128, 1024], F32, tag="acc")
                nc.vector.tensor_scalar(
                    acc[:, :nj], e_tiles[0][:, :nj], w_sb2[:, 0:1], None,
                    mybir.AluOpType.mult,
                )
                for h in range(1, H):
                    nc.vector.scalar_tensor_tensor(
                        acc[:, :nj],
                        e_tiles[h][:, :nj],
                        w_sb2[:, h : h + 1],
                        acc[:, :nj],
                        mybir.AluOpType.mult,
                        mybir.AluOpType.add,
                    )

                # links = ln(acc / sum_h ge_h)
                links = linkpool.tile([128, 1024], F32, tag="links")
                nc.scalar.activation(
                    out=links[:, :nj],
                    in_=acc[:, :nj],
                    func=A.Ln,
                    scale=rsg_sb[:, ib : ib + 1],
                )
                nc.sync.dma_start(out=out_d[i0 : i0 + 128, i0:T], in_=links[:, :nj])

    nc.finalize()
    return nc


def _chunks128(x2d):
    """[C*128, W] -> [128, C*W] with chunk c occupying cols [c*W, (c+1)*W)."""
    c = x2d.shape[0] // 128
    return np.ascontiguousarray(
        x2d.reshape(c, 128, x2d.shape[1]).transpose(1, 0, 2).reshape(128, -1)
    )


def kernel(features, pos_embed, tokens, Wq, bq, Wk, bk, Wg, bg, _trace=False):
    global LAST_RESULTS
    features = np.asarray(features, np.float32)
    pos_embed = np.asarray(pos_embed, np.float32)
    tokens = np.asarray(tokens)
    Wq = np.asarray(Wq, np.float32)
    Wk = np.asarray(Wk, np.float32)
    Wg = np.asarray(Wg, np.float32)
    bq = np.asarray(bq, np.float32)
    bk = np.asarray(bk, np.float32)
    bg = np.asarray(bg, np.float32)

    # host-side prep (sharding/layout transforms only)
    ft = np.concatenate([features, pos_embed], axis=-1)          # [B, T, 2D]
    wq_cols = _chunks128(np.ascontiguousarray(Wq.T))             # [128, 8*512]
    wk_cols = _chunks128(np.ascontiguousarray(Wk.T))
    bq4 = np.ascontiguousarray(bq.reshape(MT, 128).T)            # [128, MT]
    bk4 = np.ascontiguousarray(bk.reshape(MT, 128).T)
    # gate path on host (0.01% of FLOPs): ge = exp(f@Wg.T + bg), rsg = 1/sum_h
    gl64 = (ft @ Wg.T + bg).astype(np.float64)
    ge = np.exp(gl64).astype(np.float32)                         # [B, T, H]
    rsg = (1.0 / np.exp(gl64).sum(-1)).astype(np.float32)        # [B, T]

    n_valid = (tokens != 0).sum(axis=1).astype(np.int64)         # [B]
    w_col = T - int(n_valid.min())

    if w_col not in _NC_CACHE:
        _NC_CACHE[w_col] = _build_nc(w_col)
    nc = _NC_CACHE[w_col]

    tri = np.where(
        np.arange(128)[:, None] < np.arange(128)[None, :], 0.0, float(NEG)
    ).astype(np.float32)

    in_maps = []
    for b in range(B):
        bundle_f = np.empty((128, WF), np.float32)
        bundle_f[:, OF_FT : OF_FT + KC * T] = _chunks128(
            np.ascontiguousarray(ft[b].T)
        )
        bundle_f[:, OF_WQ : OF_WQ + KC * D] = wq_cols
        bundle_f[:, OF_WK : OF_WK + KC * D] = wk_cols
        bundle_f[:, OF_BQ : OF_BQ + MT] = bq4
        bundle_f[:, OF_BK : OF_BK + MT] = bk4
        bundle_f[:, OF_GE : OF_GE + PB * H] = (
            ge[b].reshape(PB, 128, H).transpose(1, 0, 2).reshape(128, PB * H)
        )
        bundle_f[:, OF_RSG : OF_RSG + PB] = rsg[b].reshape(PB, 128).T

        bundle_b = np.zeros((128, WB), np.float32)
        bundle_b[:, OB_ID : OB_ID + 128] = np.eye(128, dtype=np.float32)
        bundle_b[:, OB_TRI : OB_TRI + 128] = tri
        cm = np.where(np.arange(T) < n_valid[b], 0.0, float(NEG))
        bundle_b[:, OB_CM : OB_CM + T] = cm[None, :]
        bundle_b[:, OB_ONE : OB_ONE + 128] = 1.0

        in_maps.append(
            dict(
                bundle_f=bundle_f,
                bundle_b=bundle_b.astype(ml_dtypes.bfloat16),
            )
        )

    res = run_bass_kernel_spmd(nc, in_maps, core_ids=list(range(B)), trace=_trace)
    LAST_RESULTS = res

    # ---- unshard + fix all -inf positions on the host ----
    out = np.empty((B, T, T), np.float32)
    cols = np.arange(T)
    for b in range(B):
        ob = res.results[b]["out"]
        nv = int(n_valid[b])
        valid = (
            (cols[None, :] > cols[:, None])
            & (cols[None, :] < nv)
            & (cols[:, None] < nv - 1)
        )
        out[b] = np.where(valid, ob, -np.inf)
    return out


def bench_device(n_iters=32, w_col=None):
    """Time repeated device executions of the compiled module.

    Reuses the jitted shard_map callable across calls (unlike
    run_bass_kernel_spmd which re-jits), so steady-state per-call wall time
    approximates NEFF execution time + launch overhead. Returns
    (serialized_ns, pipelined_ns) per call.
    """
    import time

    import jax
    from jax.experimental.shard_map import shard_map
    from jax.sharding import Mesh, PartitionSpec

    from concourse import bass2jax
    from concourse.bass2jax import (
        _bass_exec_p,
        install_neuronx_cc_hook,
        partition_id_tensor,
    )

    if w_col is None:
        w_col = next(iter(_NC_CACHE)) if _NC_CACHE else 1
    if w_col not in _NC_CACHE:
        _NC_CACHE[w_col] = _build_nc(w_col)
    nc = _NC_CACHE[w_col]
    install_neuronx_cc_hook()

    rng = np.random.default_rng(0)
    in_names, out_names, out_avals, zero_outs = [], [], [], []
    for alloc in nc.m.functions[0].allocations:
        if not isinstance(alloc, mybir.MemoryLocationSet):
            continue
        name = alloc.memorylocations[0].name
        if alloc.kind == "ExternalInput":
            if name != (nc.partition_id_tensor.name if nc.partition_id_tensor else None):
                in_names.append((name, alloc.tensor_shape, mybir.dt.np(alloc.dtype)))
        elif alloc.kind == "ExternalOutput":
            out_names.append(name)
            out_avals.append(
                jax.core.ShapedArray(tuple(alloc.tensor_shape), mybir.dt.np(alloc.dtype))
            )
            zero_outs.append(
                np.zeros(tuple(alloc.tensor_shape), mybir.dt.np(alloc.dtype))
            )

    all_names = [n for n, _, _ in in_names] + out_names
    pid_name = nc.partition_id_tensor.name if nc.partition_id_tensor else None
    if pid_name is not None:
        all_names.append(pid_name)

    def _body(*args):
        operands = list(args)
        if pid_name is not None:
            operands.append(partition_id_tensor())
        return tuple(
            _bass_exec_p.bind(
                *operands,
                out_avals=tuple(out_avals),
                in_names=tuple(all_names),
                out_names=tuple(out_names),
                lowering_input_output_aliases=(),
                sim_require_finite=True,
                sim_require_nnan=True,
                nc=nc,
            )
        )

    devices = jax.devices()[:B]
    mesh = Mesh(np.asarray(devices), ("core",))
    nin = len(in_names) + len(zero_outs)
    sharded = jax.jit(
        shard_map(
            _body,
            mesh=mesh,
            in_specs=(PartitionSpec("core"),) * nin,
            out_specs=(PartitionSpec("core"),) * len(out_names),
            check_rep=False,
        ),
        keep_unused=True,
    )
    concat_in = [
        jax.device_put(
            np.concatenate(
                [
                    (rng.standard_normal((1, *shape)) * 0.01).astype(dt).reshape(shape)
                    for _ in range(B)
                ],
                axis=0,
            )
        )
        for _, shape, dt in in_names
    ] + [
        jax.device_put(np.zeros((B * z.shape[0], *z.shape[1:]), z.dtype))
        for z in zero_outs
    ]

    out = sharded(*concat_in)  # warmup/compile
    jax.block_until_ready(out)

    t0 = time.perf_counter()
    for _ in range(n_iters):
        out = sharded(*concat_in)
        jax.block_until_ready(out)
    ser = (time.perf_counter() - t0) / n_iters * 1e9

    t0 = time.perf_counter()
    outs = [sharded(*concat_in) for _ in range(n_iters)]
    jax.block_until_ready(outs)
    pipe = (time.perf_counter() - t0) / n_iters * 1e9

    return ser, pipe


if __name__ == "__main__":
    # smoke test with random data
    rng = np.random.default_rng(0)
    inputs = dict(
        features=rng.standard_normal((B, T, D), dtype=np.float32),
        pos_embed=rng.standard_normal((B, T, D), dtype=np.float32),
        tokens=rng.integers(0, 32000, (B, T)).astype(np.int32),
        Wq=(rng.standard_normal((D, D2), dtype=np.float32) * 0.02),
        bq=np.zeros(D, np.float32),
        Wk=(rng.standard_normal((D, D2), dtype=np.float32) * 0.02),
        bk=np.zeros(D, np.float32),
        Wg=(rng.standard_normal((H, D2), dtype=np.float32) * 0.02),
        bg=np.zeros(H, np.float32),
    )
    o = kernel(**inputs)
    print("ok", o.shape, np.isfinite(o).mean())


# revision 9
# speedup vs baseline: 1.3864x; 1.3864x over previous
"""Trainium2 Bass kernel for nn_DirectedAcyclicDecoder (sparse banded attention).

Contract: kernel(**inputs) takes FULL unsharded numpy inputs, returns the FULL
[B, T, T] float32 output. Internally shards batch across 8 NeuronCores (one
example per core) and runs a fused Bass/Tile kernel per core.

Math (per batch b, fused form — validated against the jax reference):
  f   = concat(features, pos_embed)           [T, 2D]
  q   = f @ Wq.T + bq ; k = f @ Wk.T + bk     [T, D], heads of CH=64
  ge  = exp(f @ Wg.T + bg)                    [T, H]   (unnormalized gates)
  raw_h[i,j] = q_h[i] . k_h[j]                (banded: j in (i, n_valid))
  E_h = exp(raw_h/8 + mask)  ; Z_h[i] = sum_j E_h[i,j]
  out[i,j] = ln( sum_h (ge_h[i]/Z_h[i]) * E_h[i,j] / sum_h ge_h[i] )
invalid positions are fixed to -inf on the host during unshard.

Device pipeline per core: bf16 projections (PE) -> per 128-row block: banded
QK^T scores with additive masks folded into PSUM via extra matmuls (PE),
fused exp+row-sum (ACT), head-weighted accumulation (DVE), ln (ACT), DMA out.
"""

import os
import sys

import numpy as np

for _p in ("/opt/trn_rl_repo",):
    if _p not in sys.path:
        sys.path.insert(0, _p)

import ml_dtypes  # noqa: E402

import concourse.bass as bass  # noqa: E402
import concourse.bacc as bacc  # noqa: E402
import concourse.tile as tile  # noqa: E402
from concourse import mybir  # noqa: E402
from concourse.bass_utils import run_bass_kernel_spmd  # noqa: E402

B, T, D, H, CH = 8, 1024, 512, 8, 64
D2 = 2 * D            # 1024, contraction dim of the projections
KC = D2 // 128        # 8 contraction chunks
MT = D // 128         # 4 output chunks for q/k (2 heads each)
PB = T // 128         # 8 query-position blocks of 128
NEG = np.float32(-8e30)   # additive mask on raw scores (pre /8 scale)

# bf16 weights/consts bundle column layout
OW_WQ = 0                 # wq chunks, c-major          [8 x 512]
OW_WK = OW_WQ + KC * D    # wk chunks, c-major          [8 x 512]
OW_ID = OW_WK + KC * D    # identity 128
OW_TRI = OW_ID + 128      # strict-upper-triangular additive mask (0 / NEG)
OW_CM = OW_TRI + 128      # column mask (0 / NEG), replicated on partitions
OW_ONE = OW_CM + T        # ones
WW = OW_ONE + 128

# f32 small bundle column layout
OS_BQ = 0                 # bq per-partition            [4]
OS_BK = OS_BQ + MT        # bk per-partition            [4]
OS_GE = OS_BK + MT        # exp(gate logits), ib-major  [8 x 8]
OS_RSG = OS_GE + PB * H   # 1/sum_h ge, ib-major        [8]
WS = OS_RSG + PB

F32 = mybir.dt.float32
BF16 = mybir.dt.bfloat16

_NC_CACHE: dict = {}
LAST_RESULTS = None   # BassKernelResults of the last run (for test harness)


def _force_single_act_table():
    """Restrict the activation tables so Exp/Ln/Identity/Copy resolve only in
    natural_log_exp_and_others -> exactly one ACT table load per kernel
    (instead of thrashing ~2.7us per exp<->ln switch)."""
    from concourse.bacc import get_activation_tables

    A = mybir.ActivationFunctionType
    tables = get_activation_tables("gen3")   # functools.cache'd dict
    keep = {A.Exp, A.Ln, A.Identity, A.Copy}
    for name, funcs in tables.items():
        if name != "natural_log_exp_and_others":
            funcs -= keep


def _build_nc(w_col: int) -> "bacc.Bacc":
    """Build the per-core Bass module.

    w_col: number of rightmost score columns needing the data-driven column
           mask (= T - min_b n_valid[b]); 0 compiles the mask matmul out.
    """
    _force_single_act_table()
    nc = bacc.Bacc("TRN2", target_bir_lowering=False)
    A = mybir.ActivationFunctionType

    ft_d = nc.dram_tensor("ft", [D2, T], BF16, kind="ExternalInput")
    bw_d = nc.dram_tensor("bundle_w", [128, WW], BF16, kind="ExternalInput")
    bs_d = nc.dram_tensor("bundle_s", [128, WS], F32, kind="ExternalInput")
    out_d = nc.dram_tensor("out", [T, T], F32, kind="ExternalOutput")

    with tile.TileContext(nc) as tc:
        with (
            tc.tile_pool(name="persist", bufs=1) as persist,
            tc.tile_pool(name="qk", bufs=1) as qkp,
            tc.tile_pool(name="psum_proj", bufs=2, space="PSUM") as pproj,
            tc.tile_pool(name="psum_scores", bufs=3, space="PSUM") as pscore,
            tc.tile_pool(name="epool", bufs=10) as epool,
            tc.tile_pool(name="accpool", bufs=2) as accpool,
            tc.tile_pool(name="linkpool", bufs=2) as linkpool,
            tc.tile_pool(name="small", bufs=4) as small,
        ):
            # ---- loads --------------------------------------------------
            bs_sb = persist.tile([128, WS], F32, tag="bs")
            nc.sync.dma_start(out=bs_sb, in_=bs_d[:])
            bw_sb = persist.tile([128, WW], BF16, tag="bw")
            nc.sync.dma_start(out=bw_sb, in_=bw_d[:])
            ft_sb = persist.tile([128, KC, T], BF16, tag="ft")
            ft_r = ft_d[:].rearrange("(c p) t -> p c t", p=128)
            for c in range(KC):
                nc.sync.dma_start(out=ft_sb[:, c, :], in_=ft_r[:, c, :])

            wq_sb = bw_sb[:, OW_WQ : OW_WQ + KC * D].rearrange("p (c m) -> p c m", m=D)
            wk_sb = bw_sb[:, OW_WK : OW_WK + KC * D].rearrange("p (c m) -> p c m", m=D)
            ident_sb = bw_sb[:, OW_ID : OW_ID + 128]
            tri_sb = bw_sb[:, OW_TRI : OW_TRI + 128]
            cmask_sb = bw_sb[0:1, OW_CM : OW_CM + T]
            ones1_sb = bw_sb[0:1, OW_ONE : OW_ONE + 128]
            bq_sb = bs_sb[:, OS_BQ : OS_BQ + MT]
            bk_sb = bs_sb[:, OS_BK : OS_BK + MT]
            ge_sb = bs_sb[:, OS_GE : OS_GE + PB * H].rearrange("p (b h) -> p b h", h=H)
            rsg_sb = bs_sb[:, OS_RSG : OS_RSG + PB]

            # ---- projections: qt/kt [d_out, t] = W @ f.T + b -----------
            qt_sb = qkp.tile([128, MT, T], BF16, tag="qt")
            kt_sb = qkp.tile([128, MT, T], BF16, tag="kt")
            for m in range(MT):
                for nt in range(T // 512):
                    for w_sb, o_sb, b_sb in (
                        (wq_sb, qt_sb, bq_sb),
                        (wk_sb, kt_sb, bk_sb),
                    ):
                        ps = pproj.tile([128, 512], F32, tag="proj")
                        for c in range(KC):
                            nc.tensor.matmul(
                                ps,
                                lhsT=w_sb[:, c, m * 128 : (m + 1) * 128],
                                rhs=ft_sb[:, c, nt * 512 : (nt + 1) * 512],
                                start=(c == 0),
                                stop=(c == KC - 1),
                            )
                        # PSUM -> SBUF (bf16 downcast) with per-partition bias
                        nc.vector.tensor_scalar_add(
                            o_sb[:, m, nt * 512 : (nt + 1) * 512],
                            ps,
                            b_sb[:, m : m + 1],
                        )

            # ---- per query-position block ------------------------------
            for ib in range(PB):
                i0 = ib * 128
                nj = T - i0
                e_tiles = []
                z_sb = small.tile([128, H], F32, tag="z")
                for pair in range(MT):
                    ps_a = pscore.tile([128, 1024], F32, tag="scores")
                    ps_b = pscore.tile([128, 1024], F32, tag="scores")
                    ps2 = [ps_a, ps_b]
                    for half in range(2):
                        r0 = 64 * half
                        ps = ps2[half]
                        tp = (r0, 0)
                        for j0 in range(0, nj, 512):
                            jw = min(512, nj - j0)
                            nc.tensor.matmul(
                                ps[:, j0 : j0 + jw],
                                lhsT=qt_sb[r0 : r0 + 64, pair, i0 : i0 + 128],
                                rhs=kt_sb[r0 : r0 + 64, pair, i0 + j0 : i0 + j0 + jw],
                                start=True,
                                stop=False,
                                skip_group_check=True,
                                tile_position=tp,
                            )
                    for half in range(2):
                        h = 2 * pair + half
                        ps = ps2[half]
                        # triangular mask on the diagonal 128 cols:
                        # psum[:, :128] += I.T @ trimask
                        nc.tensor.matmul(
                            ps[:, 0:128],
                            lhsT=ident_sb,
                            rhs=tri_sb,
                            start=False,
                            stop=(w_col == 0),
                            skip_group_check=True,
                        )
                        if w_col > 0:
                            # column mask for j >= min n_valid
                            nc.tensor.matmul(
                                ps[:, nj - w_col : nj],
                                lhsT=ones1_sb,
                                rhs=cmask_sb[:, T - w_col : T],
                                start=False,
                                stop=True,
                                skip_group_check=True,
                            )
                        # E = exp(raw/8) (bf16), Z = row-sum(E) fused on ACT
                        e_sb = epool.tile([128, 1024], BF16, tag="E")
                        nc.scalar.activation(
                            out=e_sb[:, :nj],
                            in_=ps[:, :nj],
                            func=A.Exp,
                            scale=0.125,
                            accum_out=z_sb[:, h : h + 1],
                        )
                        e_tiles.append(e_sb)

                # w_h = ge_h / (Z_h + eps)   [128, H]
                zr_sb = small.tile([128, H], F32, tag="zr")
                nc.vector.tensor_scalar_add(zr_sb, z_sb, 1e-30)
                nc.vector.reciprocal(zr_sb, zr_sb)
                w_sb2 = small.tile([128, H], F32, tag="w")
                nc.vector.tensor_tensor(
                    w_sb2, ge_sb[:, ib, :], zr_sb, mybir.AluOpType.mult
                )

                # acc = sum_h w_h * E_h   (bf16 chain, fp32 internal ALU)
                acc = accpool.tile([128, 1024], BF16, tag="acc")
                nc.vector.tensor_scalar(
                    acc[:, :nj], e_tiles[0][:, :nj], w_sb2[:, 0:1], None,
                    mybir.AluOpType.mult,
                )
                for h in range(1, H):
                    nc.vector.scalar_tensor_tensor(
                        acc[:, :nj],
                        e_tiles[h][:, :nj],
                        w_sb2[:, h : h + 1],
                        acc[:, :nj],
                        mybir.AluOpType.mult,
                        mybir.AluOpType.add,
                    )

                # links = ln(acc / sum_h ge_h)
                links = linkpool.tile([128, 1024], F32, tag="links")
                nc.scalar.activation(
                    out=links[:, :nj],
                    in_=acc[:, :nj],
                    func=A.Ln,
                    scale=rsg_sb[:, ib : ib + 1],
                )
                nc.sync.dma_start(out=out_d[i0 : i0 + 128, i0:T], in_=links[:, :nj])

    nc.finalize()
    return nc


def _chunks128(x2d):
    """[C*128, W] -> [128, C*W] with chunk c occupying cols [c*W, (c+1)*W)."""
    c = x2d.shape[0] // 128
    return np.ascontiguousarray(
        x2d.reshape(c, 128, x2d.shape[1]).transpose(1, 0, 2).reshape(128, -1)
    )


def kernel(features, pos_embed, tokens, Wq, bq, Wk, bk, Wg, bg, _trace=False):
    global LAST_RESULTS
    features = np.asarray(features, np.float32)
    pos_embed = np.asarray(pos_embed, np.float32)
    tokens = np.asarray(tokens)
    Wq = np.asarray(Wq, np.float32)
    Wk = np.asarray(Wk, np.float32)
    Wg = np.asarray(Wg, np.float32)
    bq = np.asarray(bq, np.float32)
    bk = np.asarray(bk, np.float32)
    bg = np.asarray(bg, np.float32)

    # host-side prep (sharding/layout transforms only)
    ft = np.concatenate([features, pos_embed], axis=-1)          # [B, T, 2D]
    wq_cols = _chunks128(np.ascontiguousarray(Wq.T)).astype(ml_dtypes.bfloat16)
    wk_cols = _chunks128(np.ascontiguousarray(Wk.T)).astype(ml_dtypes.bfloat16)
    bq4 = np.ascontiguousarray(bq.reshape(MT, 128).T)            # [128, MT]
    bk4 = np.ascontiguousarray(bk.reshape(MT, 128).T)
    # gate path on host (0.01% of FLOPs): ge = exp(f@Wg.T + bg), rsg = 1/sum_h
    gl64 = (ft @ Wg.T + bg).astype(np.float64)
    ge = np.exp(gl64).astype(np.float32)                         # [B, T, H]
    rsg = (1.0 / np.exp(gl64).sum(-1)).astype(np.float32)        # [B, T]

    n_valid = (tokens != 0).sum(axis=1).astype(np.int64)         # [B]
    w_col = T - int(n_valid.min())

    if w_col not in _NC_CACHE:
        _NC_CACHE[w_col] = _build_nc(w_col)
    nc = _NC_CACHE[w_col]

    tri = np.where(
        np.arange(128)[:, None] < np.arange(128)[None, :], 0.0, float(NEG)
    ).astype(np.float32)

    bundle_w0 = np.zeros((128, WW), np.float32)
    bundle_w0[:, OW_WQ : OW_WQ + KC * D] = wq_cols.astype(np.float32)
    bundle_w0[:, OW_WK : OW_WK + KC * D] = wk_cols.astype(np.float32)
    bundle_w0[:, OW_ID : OW_ID + 128] = np.eye(128, dtype=np.float32)
    bundle_w0[:, OW_TRI : OW_TRI + 128] = tri
    bundle_w0[:, OW_ONE : OW_ONE + 128] = 1.0

    in_maps = []
    for b in range(B):
        bw = bundle_w0.copy()
        cm = np.where(np.arange(T) < n_valid[b], 0.0, float(NEG))
        bw[:, OW_CM : OW_CM + T] = cm[None, :]

        bs = np.empty((128, WS), np.float32)
        bs[:, OS_BQ : OS_BQ + MT] = bq4
        bs[:, OS_BK : OS_BK + MT] = bk4
        bs[:, OS_GE : OS_GE + PB * H] = (
            ge[b].reshape(PB, 128, H).transpose(1, 0, 2).reshape(128, PB * H)
        )
        bs[:, OS_RSG : OS_RSG + PB] = rsg[b].reshape(PB, 128).T

        in_maps.append(
            dict(
                ft=np.ascontiguousarray(ft[b].T).astype(ml_dtypes.bfloat16),
                bundle_w=bw.astype(ml_dtypes.bfloat16),
                bundle_s=bs,
            )
        )

    res = run_bass_kernel_spmd(nc, in_maps, core_ids=list(range(B)), trace=_trace)
    LAST_RESULTS = res

    # ---- unshard + fix all -inf positions on the host ----
    out = np.empty((B, T, T), np.float32)
    cols = np.arange(T)
    for b in range(B):
        ob = res.results[b]["out"]
        nv = int(n_valid[b])
        valid = (
            (cols[None, :] > cols[:, None])
            & (cols[None, :] < nv)
            & (cols[:, None] < nv - 1)
        )
        out[b] = np.where(valid, ob, -np.inf)
    return out


def bench_device(n_iters=32, w_col=None):
    """Time repeated device executions of the compiled module.

    Reuses the jitted shard_map callable across calls (unlike
    run_bass_kernel_spmd which re-jits), so steady-state per-call wall time
    approximates NEFF execution time + launch overhead. Returns
    (serialized_ns, pipelined_ns) per call.
    """
    import time

    import jax
    from jax.experimental.shard_map import shard_map
    from jax.sharding import Mesh, PartitionSpec

    from concourse.bass2jax import (
        _bass_exec_p,
        install_neuronx_cc_hook,
        partition_id_tensor,
    )

    if w_col is None:
        w_col = next(iter(_NC_CACHE)) if _NC_CACHE else 1
    if w_col not in _NC_CACHE:
        _NC_CACHE[w_col] = _build_nc(w_col)
    nc = _NC_CACHE[w_col]
    install_neuronx_cc_hook()

    rng = np.random.default_rng(0)
    in_names, out_names, out_avals, zero_outs = [], [], [], []
    for alloc in nc.m.functions[0].allocations:
        if not isinstance(alloc, mybir.MemoryLocationSet):
            continue
        name = alloc.memorylocations[0].name
        if alloc.kind == "ExternalInput":
            if name != (nc.partition_id_tensor.name if nc.partition_id_tensor else None):
                in_names.append((name, alloc.tensor_shape, mybir.dt.np(alloc.dtype)))
        elif alloc.kind == "ExternalOutput":
            out_names.append(name)
            out_avals.append(
                jax.core.ShapedArray(tuple(alloc.tensor_shape), mybir.dt.np(alloc.dtype))
            )
            zero_outs.append(
                np.zeros(tuple(alloc.tensor_shape), mybir.dt.np(alloc.dtype))
            )

    all_names = [n for n, _, _ in in_names] + out_names
    pid_name = nc.partition_id_tensor.name if nc.partition_id_tensor else None
    if pid_name is not None:
        all_names.append(pid_name)

    def _body(*args):
        operands = list(args)
        if pid_name is not None:
            operands.append(partition_id_tensor())
        return tuple(
            _bass_exec_p.bind(
                *operands,
                out_avals=tuple(out_avals),
                in_names=tuple(all_names),
                out_names=tuple(out_names),
                lowering_input_output_aliases=(),
                sim_require_finite=True,
                sim_require_nnan=True,
                nc=nc,
            )
        )

    devices = jax.devices()[:B]
    mesh = Mesh(np.asarray(devices), ("core",))
    nin = len(in_names) + len(zero_outs)
    sharded = jax.jit(
        shard_map(
            _body,
            mesh=mesh,
            in_specs=(PartitionSpec("core"),) * nin,
            out_specs=(PartitionSpec("core"),) * len(out_names),
            check_rep=False,
        ),
        keep_unused=True,
    )
    concat_in = [
        jax.device_put(
            np.concatenate(
                [
                    (rng.standard_normal((1, *shape)) * 0.01).astype(dt).reshape(shape)
                    for _ in range(B)
                ],
                axis=0,
            )
        )
        for _, shape, dt in in_names
    ] + [
        jax.device_put(np.zeros((B * z.shape[0], *z.shape[1:]), z.dtype))
        for z in zero_outs
    ]

    out = sharded(*concat_in)  # warmup/compile
    jax.block_until_ready(out)

    t0 = time.perf_counter()
    for _ in range(n_iters):
        out = sharded(*concat_in)
        jax.block_until_ready(out)
    ser = (time.perf_counter() - t0) / n_iters * 1e9

    t0 = time.perf_counter()
    outs = [sharded(*concat_in) for _ in range(n_iters)]
    jax.block_until_ready(outs)
    pipe = (time.perf_counter() - t0) / n_iters * 1e9

    return ser, pipe


if __name__ == "__main__":
    # smoke test with random data
    rng = np.random.default_rng(0)
    inputs = dict(
        features=rng.standard_normal((B, T, D), dtype=np.float32),
        pos_embed=rng.standard_normal((B, T, D), dtype=np.float32),
        tokens=rng.integers(0, 32000, (B, T)).astype(np.int32),
        Wq=(rng.standard_normal((D, D2), dtype=np.float32) * 0.02),
        bq=np.zeros(D, np.float32),
        Wk=(rng.standard_normal((D, D2), dtype=np.float32) * 0.02),
        bk=np.zeros(D, np.float32),
        Wg=(rng.standard_normal((H, D2), dtype=np.float32) * 0.02),
        bg=np.zeros(H, np.float32),
    )
    o = kernel(**inputs)
    print("ok", o.shape, np.isfinite(o).mean())
